# revision 17
# baseline (speedup 1.0000x reference)
"""3-layer GCN (B=32 graphs, N=512 nodes, D=512 feats) on 8 trn2 NeuronCores.

Sharding: data-parallel over graphs — 4 graphs per core, weights replicated.

Math per graph g, per layer l:  h <- adj @ (h @ Wl) + bl  (relu on l=0,1).

Device layout trick: each layer is two matmuls whose contraction dims
alternate (features d, then nodes m).  We chain them with no on-device
transposes by keeping the layer input as G = H^T (feature-on-partition):
  MM1: S[n_i, e]   = sum_d G[d, n_i]^T W[d, e]     (lhsT=G chunk, rhs=W)
  MM2: G'[e_j, n]  = sum_m S[m, e_j]^T A^T[m, n]   (lhsT=S chunk, rhs=A^T)
MM2's output is already H'^T, feeding the next layer's MM1.  The host
pre-transposes batch_graph (-> X^T) and adj (-> A^T) and transposes the
final output back; those are free w.r.t. HW kernel time.

PE-warmth design (the tensor engine's HAM clock gate runs it at 1.2 GHz
until it sees ~3.4us of continuous activity, and re-throttles on idle):
  - a short warm-up matmul burst covers the initial weight/input DMA fill
  - input loads (SP queue) and output stores (DVE queue) use separate DMA
    queues so the next iteration's loads are not serialized behind stores
  - weights for layers 1/2 load on the scalar queue, concurrent with the
    first graph loads on SP, so the first matmul starts as early as possible
  - deep input pools let loads for iteration i+1 prefetch during i
"""

import numpy as np

import concourse.bass as bass
import concourse.mybir as mybir
import concourse.tile as tile
from concourse import bacc
from concourse.bass_utils import run_bass_kernel_spmd

B, N, D = 32, 512, 512
N_CORES = 8
GPC = B // N_CORES  # graphs per core
P = 128
KO = D // P  # 128-partition chunks per 512 dim

# Matmul input dtype: float32 (exact, 4 cyc/row), float32r (fast fp32 path,
# 1 cyc/row at N>=256), bfloat16 (1 cyc/row, half the DMA/SBUF footprint).
# bf16 keeps l2 error ~5e-3 (gate 2e-2) and halves input bandwidth, which
# removes most input-DMA pacing stalls.
MM_DT = mybir.dt.bfloat16

_CACHE = {}
LAST_RESULTS = None


def _build(reps=1, order="bfs", warm=12, unroll=1, xbufs=8, abufs=8, sbufs=6,
           hbufs=6, obufs=4, psum_bufs=4, wq="gpsimd", inq="sync", outq="sync"):
    f32 = mybir.dt.float32
    nc = bacc.Bacc("TRN2", target_bir_lowering=False, debug=False)

    xt = nc.dram_tensor("xt", [GPC, D, N], MM_DT, kind="ExternalInput").ap()
    at = nc.dram_tensor("at", [GPC, N, N], MM_DT, kind="ExternalInput").ap()
    w_dram = [
        nc.dram_tensor(f"w{l}", [D, D], MM_DT, kind="ExternalInput").ap()
        for l in range(3)
    ]
    b_dram = [
        nc.dram_tensor(f"b{l}", [D], f32, kind="ExternalInput").ap() for l in range(3)
    ]
    out = nc.dram_tensor("out", [GPC, D, N], f32, kind="ExternalOutput").ap()

    relu = mybir.ActivationFunctionType.Relu
    ident = mybir.ActivationFunctionType.Identity
    wq_eng = getattr(nc, wq)
    inq_eng = getattr(nc, inq)
    outq_eng = getattr(nc, outq)

    from contextlib import ExitStack

    with tile.TileContext(nc) as tc:
        with (
            tc.tile_pool(name="weights", bufs=1) as wpool,
            tc.tile_pool(name="warm", bufs=1) as wmpool,
            tc.tile_pool(name="xbuf", bufs=xbufs) as xpool,
            tc.tile_pool(name="adj", bufs=abufs) as apool,
            tc.tile_pool(name="sbuf_s", bufs=sbufs) as spool,
            tc.tile_pool(name="hbuf", bufs=hbufs) as hpool,
            tc.tile_pool(name="outp", bufs=obufs) as opool,
            tc.tile_pool(name="psum", bufs=psum_bufs, space="PSUM") as pspool,
            ExitStack() as loop_ctx,
        ):
            w_sb = [
                wpool.tile([P, KO, D], MM_DT, tag=f"w{l}", name=f"w_sb{l}")
                for l in range(3)
            ]
            b_sb = [
                wpool.tile([P, KO], f32, tag=f"b{l}", name=f"b_sb{l}")
                for l in range(3)
            ]

            def load_weights(l, eng):
                wr = w_dram[l].rearrange("(ko p) e -> p ko e", p=P)
                eng.dma_start(w_sb[l][:], wr[:])
                eng.dma_start(
                    b_sb[l][:], b_dram[l].rearrange("(ko p) -> p ko", p=P)
                )

            # w0 on the SP queue ahead of the graph loads that share it (it
            # gates the first matmuls); w1/w2 on a second queue, concurrent.
            load_weights(0, nc.sync)
            load_weights(1, wq_eng)
            load_weights(2, wq_eng)

            # PE warm-up: harmless zero matmuls covering the initial DMA fill
            # so the HAM clock gate is released by the time real work arrives.
            if warm:
                wk_l = wmpool.tile([P, P], MM_DT, tag="wk_l", name="wk_l")
                wk_r = wmpool.tile([P, N], MM_DT, tag="wk_r", name="wk_r")
                nc.vector.memset(wk_l[:], 0.0)
                nc.vector.memset(wk_r[:], 0.0)
                for wi in range(warm):
                    if wi % 4 == 0:
                        pwm = pspool.tile([P, N], f32, tag="ps", name="pwm")
                    nc.tensor.matmul(
                        pwm[:], lhsT=wk_l[:], rhs=wk_r[:],
                        start=(wi % 4 == 0), stop=(wi % 4 == 3),
                    )

            assert reps % unroll == 0
            if reps > unroll:
                loop_ctx.enter_context(tc.For_i(0, reps // unroll, 1))

            def body():
                gts, ats = [], []
                for g in range(GPC):
                    gt = xpool.tile([P, KO, N], MM_DT, tag="g", name="gt")
                    inq_eng.dma_start(
                        gt[:], xt[g].rearrange("(ko p) n -> p ko n", p=P)
                    )
                    a_t = apool.tile([P, KO, N], MM_DT, tag="a", name="a_t")
                    inq_eng.dma_start(
                        a_t[:], at[g].rearrange("(ko p) n -> p ko n", p=P)
                    )
                    gts.append(gt)
                    ats.append(a_t)

                if order == "bfs":
                    lg_order = [(l, g) for l in range(3) for g in range(GPC)]
                else:
                    lg_order = [(l, g) for g in range(GPC) for l in range(3)]
                for l, g in lg_order:
                    last = l == 2
                    gt, a_t = gts[g], ats[g]
                    # MM1: S[n_i, :] = sum_k G_k[:, n_i].T @ W_k (node-on-p.)
                    s_t = spool.tile([P, KO, D], MM_DT, tag="s", name="s_t")
                    for i in range(KO):
                        ps = pspool.tile([P, D], f32, tag="ps", name="ps")
                        for k in range(KO):
                            nc.tensor.matmul(
                                ps[:],
                                lhsT=gt[:, k, P * i : P * (i + 1)],
                                rhs=w_sb[l][:, k, :],
                                start=(k == 0),
                                stop=(k == KO - 1),
                            )
                        nc.vector.tensor_copy(s_t[:, i, :], ps[:])

                    # MM2: G'[e_j, :] = sum_k S_k[:, e_j].T @ A^T_k (feat-on-p)
                    # k-outer with all 4 psum banks open: the s_t chunk from
                    # MM1's drain of group i isn't needed until k reaches i —
                    # ~2.5us of slack vs ~0.6us for the j-outer form, so slow
                    # semaphores can't open PE micro-gaps (HAM re-throttle).
                    pool = opool if last else hpool
                    g_next = pool.tile(
                        [P, KO, N], f32 if last else MM_DT,
                        tag=("o" if last else "h"), name="g_next",
                    )
                    # Last-layer units run as two 2-bank halves so the
                    # store path overlaps the second half (shorter tail).
                    for j_blk in ([(0, 1), (2, 3)] if last else [(0, 1, 2, 3)]):
                        pzs = {
                            j: pspool.tile([P, N], f32, tag="pz", name="pz")
                            for j in j_blk
                        }
                        for k in range(KO):
                            for j in j_blk:
                                nc.tensor.matmul(
                                    pzs[j][:],
                                    lhsT=s_t[:, k, P * j : P * (j + 1)],
                                    rhs=a_t[:, k, :],
                                    start=(k == 0),
                                    stop=(k == KO - 1),
                                )
                        for j in j_blk:
                            # relu(x+b) on hidden layers, x+b on the last —
                            # both on the scalar engine; DVE does MM1 drains.
                            nc.scalar.activation(
                                g_next[:, j, :], pzs[j][:],
                                (ident if last else relu),
                                bias=b_sb[l][:, j : j + 1],
                            )
                            if last:
                                outq_eng.dma_start(
                                    out[g].rearrange(
                                        "(ko p) n -> p ko n", p=P
                                    )[:, j, :],
                                    g_next[:, j, :],
                                )
                    gts[g] = g_next

            for _ in range(unroll if reps > 1 else 1):
                body()

    nc.compile()
    return nc


def _round_f32r(x):
    """Round fp32 -> fp32r (TF32-like E8M11) on host: RNE at mantissa bit 12.

    The device fp32r memory format is an fp32 word with the low 12 mantissa
    bits zero, so pre-rounding lets the kernel DMA inputs with no cast.
    """
    b = np.ascontiguousarray(x, np.float32).view(np.uint32)
    bias = np.uint32(0x7FF) + ((b >> np.uint32(12)) & np.uint32(1))
    b = (b + bias) & np.uint32(0xFFFFF000)
    return b.view(np.float32)


def _to_mm(x):
    """Convert host fp32 -> the kernel's matmul dtype (RNE)."""
    x = np.ascontiguousarray(x, np.float32)
    if MM_DT == mybir.dt.bfloat16:
        import ml_dtypes

        return x.astype(ml_dtypes.bfloat16)
    if MM_DT == mybir.dt.float32r:
        return _round_f32r(x)
    return x


def kernel(batch_graph, adj, W0, b0, W1, b1, W2, b2, trace=False):
    global LAST_RESULTS
    if "nc" not in _CACHE:
        _CACHE["nc"] = _build()
    nc = _CACHE["nc"]

    xt = _to_mm(np.asarray(batch_graph, np.float32).transpose(0, 2, 1))
    at = _to_mm(np.asarray(adj, np.float32).transpose(0, 2, 1))
    ws = [_to_mm(np.asarray(w, np.float32)) for w in (W0, W1, W2)]
    bs = [np.ascontiguousarray(np.asarray(b, np.float32)) for b in (b0, b1, b2)]

    in_maps = []
    for c in range(N_CORES):
        sl = slice(c * GPC, (c + 1) * GPC)
        in_maps.append(
            {
                "xt": np.ascontiguousarray(xt[sl]),
                "at": np.ascontiguousarray(at[sl]),
                "w0": ws[0], "b0": bs[0],
                "w1": ws[1], "b1": bs[1],
                "w2": ws[2], "b2": bs[2],
            }
        )

    try:
        res = run_bass_kernel_spmd(
            nc, in_maps, core_ids=list(range(N_CORES)), trace=trace
        )
    except ModuleNotFoundError:
        # Tracing was requested (arg or BASS_TRACE env) but this environment
        # lacks the axon NTFF profile hook; rerun without the trace path.
        import os

        os.environ["BASS_NEVER_TRACE"] = "1"
        try:
            res = run_bass_kernel_spmd(
                nc, in_maps, core_ids=list(range(N_CORES)), trace=False
            )
        finally:
            del os.environ["BASS_NEVER_TRACE"]
    LAST_RESULTS = res
    outs = [r["out"].transpose(0, 2, 1) for r in res.results]  # [GPC, N, D] each
    return np.ascontiguousarray(np.concatenate(outs, axis=0), dtype=np.float32)


# revision 20
# speedup vs baseline: 2.1825x; 2.1825x over previous
"""3-layer GCN (B=32 graphs, N=512 nodes, D=512 feats) on 8 trn2 NeuronCores.

Sharding: data-parallel over graphs — 4 graphs per core, weights replicated.

Math per graph g, per layer l:  h <- adj @ (h @ Wl) + bl  (relu on l=0,1).

Device layout trick: each layer is two matmuls whose contraction dims
alternate (features d, then nodes m).  We chain them with no on-device
transposes by keeping the layer input as G = H^T (feature-on-partition):
  MM1: S[n_i, e]   = sum_d G[d, n_i]^T W[d, e]     (lhsT=G chunk, rhs=W)
  MM2: G'[e_j, n]  = sum_m S[m, e_j]^T A^T[m, n]   (lhsT=S chunk, rhs=A^T)
MM2's output is already H'^T, feeding the next layer's MM1.  The host
pre-transposes batch_graph (-> X^T) and adj (-> A^T) and transposes the
final output back; those are free w.r.t. HW kernel time.

PE-warmth design (the tensor engine's HAM clock gate runs it at 1.2 GHz
until it sees ~3.4us of continuous activity, and re-throttles on idle):
  - a short warm-up matmul burst covers the initial weight/input DMA fill
  - input loads (SP queue) and output stores (DVE queue) use separate DMA
    queues so the next iteration's loads are not serialized behind stores
  - weights for layers 1/2 load on the scalar queue, concurrent with the
    first graph loads on SP, so the first matmul starts as early as possible
  - deep input pools let loads for iteration i+1 prefetch during i
"""

import numpy as np

import concourse.bass as bass
import concourse.mybir as mybir
import concourse.tile as tile
from concourse import bacc
from concourse.bass_utils import run_bass_kernel_spmd

B, N, D = 32, 512, 512
N_CORES = 8
GPC = B // N_CORES  # graphs per core
P = 128
KO = D // P  # 128-partition chunks per 512 dim

# Matmul input dtype: float32 (exact, 4 cyc/row), float32r (fast fp32 path,
# 1 cyc/row at N>=256), bfloat16 (1 cyc/row, half the DMA/SBUF footprint).
# bf16 keeps l2 error ~5e-3 (gate 2e-2) and halves input bandwidth, which
# removes most input-DMA pacing stalls.
MM_DT = mybir.dt.bfloat16

_CACHE = {}
LAST_RESULTS = None


def _build(reps=1, order="bfs", warm=13, unroll=1, xbufs=8, abufs=8, sbufs=6,
           hbufs=6, obufs=4, psum_bufs=4, wq="scalar", inq="sync", outq="sync"):
    f32 = mybir.dt.float32
    nc = bacc.Bacc("TRN2", target_bir_lowering=False, debug=False)

    xt = nc.dram_tensor("xt", [GPC, D, N], MM_DT, kind="ExternalInput").ap()
    at = nc.dram_tensor("at", [GPC, N, N], MM_DT, kind="ExternalInput").ap()
    w_dram = [
        nc.dram_tensor(f"w{l}", [D, D], MM_DT, kind="ExternalInput").ap()
        for l in range(3)
    ]
    b_dram = [
        nc.dram_tensor(f"b{l}", [D], f32, kind="ExternalInput").ap() for l in range(3)
    ]
    out = nc.dram_tensor("out", [GPC, D, N], f32, kind="ExternalOutput").ap()

    relu = mybir.ActivationFunctionType.Relu
    ident = mybir.ActivationFunctionType.Identity
    wq_eng = getattr(nc, wq)
    inq_eng = getattr(nc, inq)
    outq_eng = getattr(nc, outq)

    from contextlib import ExitStack

    with tile.TileContext(nc) as tc:
        with (
            tc.tile_pool(name="weights", bufs=1) as wpool,
            tc.tile_pool(name="warm", bufs=1) as wmpool,
            tc.tile_pool(name="xbuf", bufs=xbufs) as xpool,
            tc.tile_pool(name="adj", bufs=abufs) as apool,
            tc.tile_pool(name="sbuf_s", bufs=sbufs) as spool,
            tc.tile_pool(name="hbuf", bufs=hbufs) as hpool,
            tc.tile_pool(name="outp", bufs=obufs) as opool,
            tc.tile_pool(name="psum", bufs=psum_bufs, space="PSUM") as pspool,
            ExitStack() as loop_ctx,
        ):
            w_sb = [
                wpool.tile([P, KO, D], MM_DT, tag=f"w{l}", name=f"w_sb{l}")
                for l in range(3)
            ]
            b_sb = [
                wpool.tile([P, KO], f32, tag=f"b{l}", name=f"b_sb{l}")
                for l in range(3)
            ]

            def load_weights(l, eng):
                wr = w_dram[l].rearrange("(ko p) e -> p ko e", p=P)
                eng.dma_start(w_sb[l][:], wr[:])
                eng.dma_start(
                    b_sb[l][:], b_dram[l].rearrange("(ko p) -> p ko", p=P)
                )

            # w0 on the SP queue ahead of the graph loads that share it (it
            # gates the first matmuls); w1/w2 on a second queue, concurrent.
            load_weights(0, nc.sync)
            load_weights(1, wq_eng)
            load_weights(2, wq_eng)

            # PE warm-up: harmless zero matmuls covering the initial DMA fill
            # so the HAM clock gate is released by the time real work arrives.
            if warm:
                wk_l = wmpool.tile([P, P], MM_DT, tag="wk_l", name="wk_l")
                wk_r = wmpool.tile([P, N], MM_DT, tag="wk_r", name="wk_r")
                nc.vector.memset(wk_l[:], 0.0)
                nc.vector.memset(wk_r[:], 0.0)
                for wi in range(warm):
                    if wi % 4 == 0:
                        pwm = pspool.tile([P, N], f32, tag="ps", name="pwm")
                    nc.tensor.matmul(
                        pwm[:], lhsT=wk_l[:], rhs=wk_r[:],
                        start=(wi % 4 == 0),
                        stop=(wi % 4 == 3 or wi == warm - 1),
                    )

            assert reps % unroll == 0
            if reps > unroll:
                loop_ctx.enter_context(tc.For_i(0, reps // unroll, 1))

            def body():
                gts, ats = [], []
                for g in range(GPC):
                    gt = xpool.tile([P, KO, N], MM_DT, tag="g", name="gt")
                    inq_eng.dma_start(
                        gt[:], xt[g].rearrange("(ko p) n -> p ko n", p=P)
                    )
                    a_t = apool.tile([P, KO, N], MM_DT, tag="a", name="a_t")
                    inq_eng.dma_start(
                        a_t[:], at[g].rearrange("(ko p) n -> p ko n", p=P)
                    )
                    gts.append(gt)
                    ats.append(a_t)

                if order == "bfs":
                    lg_order = [(l, g) for l in range(3) for g in range(GPC)]
                else:
                    lg_order = [(l, g) for g in range(GPC) for l in range(3)]
                for l, g in lg_order:
                    last = l == 2
                    gt, a_t = gts[g], ats[g]
                    # MM1: S[n_i, :] = sum_k G_k[:, n_i].T @ W_k (node-on-p.)
                    s_t = spool.tile([P, KO, D], MM_DT, tag="s", name="s_t")
                    for i in range(KO):
                        ps = pspool.tile([P, D], f32, tag="ps", name="ps")
                        for k in range(KO):
                            nc.tensor.matmul(
                                ps[:],
                                lhsT=gt[:, k, P * i : P * (i + 1)],
                                rhs=w_sb[l][:, k, :],
                                start=(k == 0),
                                stop=(k == KO - 1),
                            )
                        nc.vector.tensor_copy(s_t[:, i, :], ps[:])

                    # MM2: G'[e_j, :] = sum_k S_k[:, e_j].T @ A^T_k (feat-on-p)
                    # k-outer with all 4 psum banks open: the s_t chunk from
                    # MM1's drain of group i isn't needed until k reaches i —
                    # ~2.5us of slack vs ~0.6us for the j-outer form, so slow
                    # semaphores can't open PE micro-gaps (HAM re-throttle).
                    pool = opool if last else hpool
                    g_next = pool.tile(
                        [P, KO, N], f32 if last else MM_DT,
                        tag=("o" if last else "h"), name="g_next",
                    )
                    # Last-layer units run as two 2-bank halves so the
                    # store path overlaps the second half (shorter tail).
                    for j_blk in ([(0, 1), (2, 3)] if last else [(0, 1, 2, 3)]):
                        pzs = {
                            j: pspool.tile([P, N], f32, tag="pz", name="pz")
                            for j in j_blk
                        }
                        for k in range(KO):
                            for j in j_blk:
                                nc.tensor.matmul(
                                    pzs[j][:],
                                    lhsT=s_t[:, k, P * j : P * (j + 1)],
                                    rhs=a_t[:, k, :],
                                    start=(k == 0),
                                    stop=(k == KO - 1),
                                )
                        for j in j_blk:
                            # relu(x+b) on hidden layers, x+b on the last —
                            # both on the scalar engine; DVE does MM1 drains.
                            nc.scalar.activation(
                                g_next[:, j, :], pzs[j][:],
                                (ident if last else relu),
                                bias=b_sb[l][:, j : j + 1],
                            )
                            if last:
                                outq_eng.dma_start(
                                    out[g].rearrange(
                                        "(ko p) n -> p ko n", p=P
                                    )[:, j, :],
                                    g_next[:, j, :],
                                )
                    gts[g] = g_next

            for _ in range(unroll if reps > 1 else 1):
                body()

    nc.compile()
    return nc


def _round_f32r(x):
    """Round fp32 -> fp32r (TF32-like E8M11) on host: RNE at mantissa bit 12.

    The device fp32r memory format is an fp32 word with the low 12 mantissa
    bits zero, so pre-rounding lets the kernel DMA inputs with no cast.
    """
    b = np.ascontiguousarray(x, np.float32).view(np.uint32)
    bias = np.uint32(0x7FF) + ((b >> np.uint32(12)) & np.uint32(1))
    b = (b + bias) & np.uint32(0xFFFFF000)
    return b.view(np.float32)


def _to_mm(x):
    """Convert host fp32 -> the kernel's matmul dtype (RNE)."""
    x = np.ascontiguousarray(x, np.float32)
    if MM_DT == mybir.dt.bfloat16:
        import ml_dtypes

        return x.astype(ml_dtypes.bfloat16)
    if MM_DT == mybir.dt.float32r:
        return _round_f32r(x)
    return x


def kernel(batch_graph, adj, W0, b0, W1, b1, W2, b2, trace=False):
    global LAST_RESULTS
    if "nc" not in _CACHE:
        _CACHE["nc"] = _build()
    nc = _CACHE["nc"]

    xt = _to_mm(np.asarray(batch_graph, np.float32).transpose(0, 2, 1))
    at = _to_mm(np.asarray(adj, np.float32).transpose(0, 2, 1))
    ws = [_to_mm(np.asarray(w, np.float32)) for w in (W0, W1, W2)]
    bs = [np.ascontiguousarray(np.asarray(b, np.float32)) for b in (b0, b1, b2)]

    in_maps = []
    for c in range(N_CORES):
        sl = slice(c * GPC, (c + 1) * GPC)
        in_maps.append(
            {
                "xt": np.ascontiguousarray(xt[sl]),
                "at": np.ascontiguousarray(at[sl]),
                "w0": ws[0], "b0": bs[0],
                "w1": ws[1], "b1": bs[1],
                "w2": ws[2], "b2": bs[2],
            }
        )

    try:
        res = run_bass_kernel_spmd(
            nc, in_maps, core_ids=list(range(N_CORES)), trace=trace
        )
    except ModuleNotFoundError:
        # Tracing was requested (arg or BASS_TRACE env) but this environment
        # lacks the axon NTFF profile hook; rerun without the trace path.
        import os

        os.environ["BASS_NEVER_TRACE"] = "1"
        try:
            res = run_bass_kernel_spmd(
                nc, in_maps, core_ids=list(range(N_CORES)), trace=False
            )
        finally:
            del os.environ["BASS_NEVER_TRACE"]
    LAST_RESULTS = res
    outs = [r["out"].transpose(0, 2, 1) for r in res.results]  # [GPC, N, D] each
    return np.ascontiguousarray(np.concatenate(outs, axis=0), dtype=np.float32)
